# revision 5
# baseline (speedup 1.0000x reference)
"""BasketEmbedding Trainium2 kernel (Bass/Tile, 8 NeuronCores, SPMD).

Reference semantics (B=1024, S=50, M=20, H=128, table 100001x128 f32,
padding_idx = 100000 whose row is zero):

    emb    = table[item_ids]                             # [B,S,M,H]
    summed = sum over m < basket_lens[b,s] of emb        # [B,S,H]
    pooled = summed / basket_lens                        # mean pool
    out    = where(s < seq_lens[b], pooled, 100000.0)    # [B,S,H]

Strategy: data-parallel over batch. Each of the 8 cores handles 128
batches; partition p = local batch, group g = sequence position s.
Per group one indirect DMA gathers the 128x20 embedding rows (512 B
each) straight from the DRAM table into SBUF (out-of-basket item slots
are first remapped on-device to the zero padding row), a DVE
tensor_reduce sums the 20 rows, and a fused tensor_scalar applies
1/len and the sequence-position mask in one pass.
"""

import numpy as np

import concourse.bass as bass
import concourse.mybir as mybir
import concourse.tile as tile
from concourse.bass_utils import run_bass_kernel_spmd

N_CORES = 8


def _split_multi_waits(nc):
    """Walrus on this stack rejects >1 sync-wait command per instruction
    ("Too many sync wait commands", CoreV3GenImpl setupSyncWait). Tile
    freely attaches several SyncWaits to one instruction, so hoist all
    but the last wait of each instruction onto same-engine NoOps
    inserted directly before it — identical sequencer semantics.
    """
    fn = nc.m.functions[0]
    for bb in fn.blocks:
        insts = bb.instructions
        if not any(i.sync_info and i.sync_info.on_wait
                   and len(i.sync_info.on_wait) > 1 for i in insts):
            continue
        new_list = []
        for inst in insts:
            si = inst.sync_info
            if si is not None and si.on_wait and len(si.on_wait) > 1:
                waits = list(si.on_wait)
                for k, w in enumerate(waits[:-1]):
                    nop = mybir.InstNoOp(name=f"{inst.name}-w{k}", ins=[],
                                         outs=[])
                    nop.engine = inst.engine
                    nop.sync_info = mybir.SyncInfo(on_wait=[w], on_update=[])
                    new_list.append(nop)
                inst.sync_info = mybir.SyncInfo(
                    on_wait=[waits[-1]],
                    on_update=list(si.on_update) if si.on_update else [])
            new_list.append(inst)
        bb.instructions = new_list


P = 128        # SBUF partitions = batches per core (1024 / 8)
S = 50         # sequence positions (= groups)
M = 20         # max items per basket
H = 128        # hidden size
NROWS = 100001
PAD_ID = 100000
PAD_VAL = 100000.0

F32 = mybir.dt.float32
I32 = mybir.dt.int32
OP = mybir.AluOpType


def build_nc(s=S, m=M, h=H, nrows=NROWS, pad_id=PAD_ID, pad_val=PAD_VAL,
             gather_bufs=4):
    nc = bass.Bass()

    table = nc.dram_tensor("table", [nrows, h], F32, kind="ExternalInput").ap()
    ids = nc.dram_tensor("ids", [P, s * m], I32, kind="ExternalInput").ap()
    lens = nc.dram_tensor("lens", [P, s], I32, kind="ExternalInput").ap()
    slen = nc.dram_tensor("slen", [P, 1], I32, kind="ExternalInput").ap()
    out = nc.dram_tensor("out", [P, s, h], F32, kind="ExternalOutput").ap()

    with tile.TileContext(nc) as tc:
        with (
            tc.tile_pool(name="const", bufs=1) as cpool,
            tc.tile_pool(name="gather", bufs=gather_bufs) as gpool,
            tc.tile_pool(name="acc", bufs=4) as apool,
            tc.tile_pool(name="fin", bufs=4) as fpool,
        ):
            ids_t = cpool.tile([P, s * m], I32, tag="ids")
            nc.sync.dma_start(ids_t[:], ids)
            lens_t = cpool.tile([P, s], I32, tag="lens")
            nc.sync.dma_start(lens_t[:], lens)
            slen_t = cpool.tile([P, 1], I32, tag="slen")
            nc.sync.dma_start(slen_t[:], slen)

            # miota[p, g*m + j] = j   (item slot index within basket)
            miota = cpool.tile([P, s * m], I32, tag="miota")
            nc.gpsimd.iota(miota[:], pattern=[[0, s], [1, m]], base=0,
                           channel_multiplier=0)
            # giota[p, g] = g         (sequence position)
            giota = cpool.tile([P, s], I32, tag="giota")
            nc.gpsimd.iota(giota[:], pattern=[[1, s]], base=0,
                           channel_multiplier=0)

            # Masked ids: slots past the basket length -> padding row
            # (whose embedding is all zeros):
            #   id' = max(id, (j >= len) * pad_id)
            pm = cpool.tile([P, s * m], I32, tag="pm")
            nc.vector.tensor_tensor(
                out=pm[:], in0=miota[:],
                in1=lens_t[:].broadcast_to([P, s, m]), op=OP.is_ge)
            nc.vector.tensor_scalar(
                out=pm[:], in0=pm[:], scalar1=pad_id, scalar2=None,
                op0=OP.mult)
            mid_t = cpool.tile([P, s * m], I32, tag="mid")
            nc.vector.tensor_tensor(
                out=mid_t[:], in0=ids_t[:], in1=pm[:], op=OP.max)

            # Per-(p,g) fused epilogue coefficients:
            #   valid  (g <  seq_len): out = acc * (1/len) + 0
            #   padded (g >= seq_len): out = acc * 0       + pad_val
            lens_f = cpool.tile([P, s], F32, tag="lensf")
            nc.vector.tensor_copy(out=lens_f[:], in_=lens_t[:])
            recip = cpool.tile([P, s], F32, tag="recip")
            nc.vector.reciprocal(recip[:], lens_f[:])
            smask = cpool.tile([P, s], F32, tag="smask")
            nc.vector.tensor_tensor(
                out=smask[:], in0=giota[:],
                in1=slen_t[:].to_broadcast([P, s]), op=OP.is_lt)
            scale = cpool.tile([P, s], F32, tag="scale")
            nc.vector.tensor_tensor(
                out=scale[:], in0=smask[:], in1=recip[:], op=OP.mult)
            offs = cpool.tile([P, s], F32, tag="offs")
            nc.vector.tensor_scalar(
                out=offs[:], in0=smask[:], scalar1=-pad_val, scalar2=pad_val,
                op0=OP.mult, op1=OP.add)

            for g in range(s):
                gt = gpool.tile([P, m * h], F32, tag="gt")
                nc.gpsimd.indirect_dma_start(
                    out=gt[:], out_offset=None,
                    in_=table,
                    in_offset=bass.IndirectOffsetOnAxis(
                        ap=mid_t[:, g * m:(g + 1) * m], axis=0),
                )
                acc = apool.tile([P, h], F32, tag="acc")
                nc.vector.tensor_reduce(
                    out=acc[:],
                    in_=gt[:].rearrange("p (m h) -> p h m", m=m),
                    axis=mybir.AxisListType.X, op=OP.add)
                ft = fpool.tile([P, h], F32, tag="ft")
                nc.vector.tensor_scalar(
                    out=ft[:], in0=acc[:],
                    scalar1=scale[:, g:g + 1], scalar2=offs[:, g:g + 1],
                    op0=OP.mult, op1=OP.add)
                nc.sync.dma_start(out[:, g, :], ft[:])

    _split_multi_waits(nc)
    return nc


_NC_CACHE = None


def kernel(table, item_ids, basket_lens, seq_lens):
    global _NC_CACHE
    table = np.ascontiguousarray(np.asarray(table), dtype=np.float32)
    ids = np.ascontiguousarray(np.asarray(item_ids)).astype(np.int32)
    lens = np.ascontiguousarray(np.asarray(basket_lens)).astype(np.int32)
    slens = np.ascontiguousarray(np.asarray(seq_lens)).astype(np.int32)

    B = ids.shape[0]
    bpc = B // N_CORES
    assert bpc == P and ids.shape == (B, S, M) and lens.shape == (B, S)

    if _NC_CACHE is None:
        _NC_CACHE = build_nc()
    nc = _NC_CACHE

    in_maps = []
    for c in range(N_CORES):
        sl = slice(c * bpc, (c + 1) * bpc)
        in_maps.append({
            "table": table,
            "ids": np.ascontiguousarray(ids[sl].reshape(bpc, S * M)),
            "lens": np.ascontiguousarray(lens[sl]),
            "slen": np.ascontiguousarray(slens[sl].reshape(bpc, 1)),
        })

    res = run_bass_kernel_spmd(nc, in_maps, list(range(N_CORES)))
    outs = [res.results[c]["out"].reshape(bpc, S, H) for c in range(N_CORES)]
    return np.concatenate(outs, axis=0).astype(np.float32, copy=False)
